# revision 8
# baseline (speedup 1.0000x reference)
"""Trainium2 Bass kernel for nn_ConceptLayer (B=8, S=2048, E=128).

out[b,s,c] = LN( einsum('sa,sp,cap->sc', h[b], s_seq[b], W) + h[b] )
  h = x @ dense_w + dense_b
  s_seq = decayed prefix sum of h along s (s_i = (s_{i-1}+h_{i-1})/1.2)

Sharding: data-parallel over batch, one sample per NeuronCore (8 cores).

One-pass PE design: outT[c,t] = sum_a W2[:,a-slice]^T @ (sT * h[:,a]) is
accumulated directly in PSUM across all 128 a-matmuls (plus one identity
matmul adding the residual h^T). No big Y intermediate, no per-a vector
combine. Per a-batch of 4 (x 1024-token supergroup):
  1. flatten h^T rows to one partition, then one stride-0 SBUF DMA per
     64 dst partitions replicates h[t,a] (bf16) to all 128 partitions
  2. DVE builds scaled operands sc[p,(a,t)] = sT[p,t]*hB[p,(a,t)]
     (16-bit tensor_tensor, stride-0 a-dim on the sT operand, 2x mode)
  3. PE: per a one W2-slice load + 2 matmuls (N=512) accumulating into
     per-token-group PSUM banks (alternating banks, no same-bank stall)
  4. per 128-token block: PE transpose back to [t,c] + LayerNorm
     (stats on DVE, sqrt+apply on ACT, gamma/beta on GPSIMD)
Scan stays fp16 (decay 1/1.2 bounds the rounding-error accumulation).
"""

import dataclasses
import os
import sys

import numpy as np

for _p in ("/opt/trn_rl_repo",):
    if _p not in sys.path and os.path.isdir(_p):
        sys.path.insert(0, _p)

import concourse.bass as bass
import concourse.bacc as bacc
import concourse.tile as tile
from concourse import mybir
from concourse.bass_utils import run_bass_kernel_spmd

B, S, E = 8, 2048, 128
DECAY = 1.2
LN_EPS = 1e-3
NSG = 2            # token supergroups
SGW = S // NSG     # 1024 tokens each
NTGL = SGW // 512  # 2 psum groups of 512 per supergroup
NAB = 32           # a-batches of 4
ABW = E // NAB     # 4
F32 = mybir.dt.float32
F16 = mybir.dt.float16
BF16 = mybir.dt.bfloat16
NPF16 = mybir.dt.np(F16)
NPBF16 = mybir.dt.np(BF16)

_CACHE = {}
LAST_RESULT = None  # BassKernelResults of the most recent run (for test.py)


def _build_nc():
    nc = bacc.Bacc(None, target_bir_lowering=False)

    xT_d = nc.declare_dram_parameter("xT", [E, S], BF16, isOutput=False)
    dw_d = nc.declare_dram_parameter("dense_w", [E, E], BF16, isOutput=False)
    bc_d = nc.declare_dram_parameter("b_col", [E, 1], F32, isOutput=False)
    bcd_d = nc.declare_dram_parameter("b_col_d", [E, 1], F32, isOutput=False)
    w2_d = nc.declare_dram_parameter("w2", [E, E * E], BF16, isOutput=False)
    id_d = nc.declare_dram_parameter("id128", [E, E], BF16, isOutput=False)
    gam_d = nc.declare_dram_parameter("gamma_rep", [128, E], F32, isOutput=False)
    bet_d = nc.declare_dram_parameter("beta_rep", [128, E], F32, isOutput=False)
    out_d = nc.declare_dram_parameter("out", [S, E], F32, isOutput=True)

    with tile.TileContext(nc) as tc:
        with (
            tc.tile_pool(name="singles", bufs=1) as singles,
            tc.tile_pool(name="flat", bufs=4) as flat_pool,
            tc.tile_pool(name="hb", bufs=2) as hb_pool,
            tc.tile_pool(name="sc", bufs=2) as sc_pool,
            tc.tile_pool(name="ln", bufs=2) as ln_pool,
            tc.tile_pool(name="small", bufs=4) as small,
            tc.tile_pool(name="h_ps", bufs=2, space="PSUM") as h_ps,
            tc.tile_pool(name="mm_ps", bufs=2, space="PSUM") as mm_ps,
            tc.tile_pool(name="tr_ps", bufs=2, space="PSUM") as tr_ps,
        ):
            # ---- resident tensors ----
            w2_sb = singles.tile([E, E * E], BF16)
            nc.sync.dma_start(out=w2_sb[:], in_=w2_d[:])
            dw_sb = singles.tile([E, E], BF16)
            nc.sync.dma_start(out=dw_sb[:], in_=dw_d[:])
            id_sb = singles.tile([E, E], BF16)
            nc.sync.dma_start(out=id_sb[:], in_=id_d[:])
            bcol = singles.tile([E, 1], F32)
            nc.sync.dma_start(out=bcol[:], in_=bc_d[:])
            bcold = singles.tile([E, 1], F32)
            nc.sync.dma_start(out=bcold[:], in_=bcd_d[:])
            gam_sb = singles.tile([128, E], F32)
            nc.sync.dma_start(out=gam_sb[:], in_=gam_d[:])
            bet_sb = singles.tile([128, E], F32)
            nc.sync.dma_start(out=bet_sb[:], in_=bet_d[:])
            xT = singles.tile([E, S], BF16)
            nc.sync.dma_start(out=xT[:], in_=xT_d[:])

            eps_t = singles.tile([128, 1], F32)
            nc.vector.memset(eps_t[:], LN_EPS)
            dinv = singles.tile([128, 512], F16)
            nc.vector.memset(dinv[:], 1.0 / DECAY)

            hTc = singles.tile([E, S], BF16)     # h^T (with bias), clean
            hTp = singles.tile([E, S + 1], F16)  # scan feed: col j+1 = (h_j+b)/d
            sTmm = singles.tile([E, S], F16)     # decayed prefix sums s^T
            nc.vector.memset(hTp[:, 0:1], 0.0)

            # ---- phase A: hT = dw^T @ xT (+bias); scan feed ----
            for q in range(4):
                lo, hi = q * 512, (q + 1) * 512
                hp = h_ps.tile([E, 512], F32)
                nc.tensor.matmul(hp[:], dw_sb[:], xT[:, lo:hi],
                                 start=True, stop=True)
                nc.vector.tensor_scalar(
                    hTc[:, lo:hi], hp[:], bcol[:], None, mybir.AluOpType.add)
                nc.scalar.activation(
                    hTp[:, lo + 1:hi + 1], hp[:],
                    mybir.ActivationFunctionType.Identity,
                    bias=bcold[:], scale=1.0 / DECAY)

            # ---- phase B: decay prefix scan -> sT (fp16) ----
            for q in range(4):
                lo, hi = q * 512, (q + 1) * 512
                init = 0.0 if q == 0 else sTmm[:, lo - 1:lo]
                nc.vector.tensor_tensor_scan(
                    sTmm[:, lo:hi], dinv[:], hTp[:, lo:hi], init,
                    mybir.AluOpType.mult, mybir.AluOpType.add)

            # ---- main: two supergroups of 1024 tokens ----
            for sg in range(NSG):
                slo = sg * SGW
                psw = mm_ps.tile([E, SGW], F32, name="psw")
                ps = [psw[:, t * 512:(t + 1) * 512] for t in range(NTGL)]
                # residual starts each accumulation: outT += Id^T @ hT
                for tgl in range(NTGL):
                    c0 = slo + tgl * 512
                    nc.tensor.matmul(ps[tgl], id_sb[:],
                                     hTc[:, c0:c0 + 512],
                                     start=True, stop=False,
                                     skip_group_check=True)
                for ab in range(NAB):
                    a0 = ab * ABW
                    # flatten 4 h^T rows (sg's 1024 cols) into one partition
                    flat = flat_pool.tile([1, ABW * SGW], BF16)
                    nc.sync.dma_start(
                        out=flat[:],
                        in_=hTc[a0:a0 + ABW, slo:slo + SGW])
                    # replicate to all 128 partitions (2 big stride-0 DMAs)
                    hB = hb_pool.tile([128, ABW, SGW], BF16)
                    fsrc = flat[:]
                    for j in range(2):
                        bsrc = dataclasses.replace(
                            fsrc, ap=[fsrc.ap[0], [0, 64], [1, ABW * SGW]])
                        nc.sync.dma_start(out=hB[64 * j:64 * (j + 1), :, :],
                                          in_=bsrc)
                    # scaled operands: sc[p,(a,t)] = sT[p,t] * h[t,a]
                    sc = sc_pool.tile([128, ABW, SGW], BF16)
                    in0 = sTmm[:, slo:slo + SGW]
                    in0 = dataclasses.replace(
                        in0, ap=[in0.ap[0], [0, ABW]] + in0.ap[1:])
                    nc.vector.tensor_tensor(out=sc[:, :, :], in0=in0,
                                            in1=hB[:, :, :],
                                            op=mybir.AluOpType.mult)
                    # PE: per a, one W2 slice x NTGL matmuls (alt. banks)
                    for i in range(ABW):
                        a = a0 + i
                        for tgl in range(NTGL):
                            last = (ab == NAB - 1) and (i == ABW - 1)
                            nc.tensor.matmul(
                                ps[tgl], w2_sb[:, a * E:(a + 1) * E],
                                sc[:, i, tgl * 512:(tgl + 1) * 512],
                                start=False, stop=last,
                                skip_group_check=True)

                # drain, transpose back, LayerNorm per 128-token block
                for tgl in range(NTGL):
                    rT = ln_pool.tile([E, 512], BF16)
                    nc.scalar.copy(out=rT[:], in_=ps[tgl])
                    for blk in range(4):
                        g = sg * (SGW // 128) + tgl * 4 + blk
                        rtp = tr_ps.tile([128, 128], BF16)
                        nc.tensor.transpose(
                            rtp[:], rT[:, blk * 128:(blk + 1) * 128], id_sb[:])
                        stats = small.tile([128, 6], F32)
                        nc.vector.bn_stats(out=stats[:], in_=rtp[:])
                        mv = small.tile([128, 2], F32)
                        nc.vector.bn_aggr(out=mv[:], in_=stats[:])
                        std = small.tile([128, 1], F32)
                        nc.scalar.activation(
                            out=std[:], in_=mv[:, 1:2],
                            func=mybir.ActivationFunctionType.Sqrt,
                            bias=eps_t[:], scale=1.0)
                        rstd = small.tile([128, 1], F32)
                        nc.vector.reciprocal(out=rstd[:], in_=std[:])
                        nbias = small.tile([128, 1], F32)
                        nc.vector.tensor_scalar(
                            nbias[:], mv[:, 0:1], rstd[:], -1.0,
                            mybir.AluOpType.mult, mybir.AluOpType.mult)
                        nrm = ln_pool.tile([128, E], F32)
                        nc.scalar.activation(
                            out=nrm[:], in_=rtp[:],
                            func=mybir.ActivationFunctionType.Identity,
                            bias=nbias[:], scale=rstd[:])
                        nc.gpsimd.tensor_tensor(out=nrm[:], in0=nrm[:],
                                                in1=gam_sb[:],
                                                op=mybir.AluOpType.mult)
                        nc.gpsimd.tensor_tensor(out=nrm[:], in0=nrm[:],
                                                in1=bet_sb[:],
                                                op=mybir.AluOpType.add)
                        nc.sync.dma_start(
                            out=out_d[g * 128:(g + 1) * 128, :], in_=nrm[:])

    nc.compile()
    return nc


def _get_nc():
    if "nc" not in _CACHE:
        _CACHE["nc"] = _build_nc()
    return _CACHE["nc"]


def kernel(x, dense_w, dense_b, concept_map, ln_gamma, ln_beta):
    global LAST_RESULT
    x = np.asarray(x, dtype=np.float32)
    dense_w = np.ascontiguousarray(np.asarray(dense_w, dtype=np.float32))
    b = np.asarray(dense_b, dtype=np.float32)
    w2 = np.ascontiguousarray(
        np.transpose(np.asarray(concept_map, dtype=np.float32), (2, 1, 0))
    ).reshape(E, E * E).astype(NPBF16)
    gam = np.ascontiguousarray(
        np.broadcast_to(np.asarray(ln_gamma, np.float32), (128, E)))
    bet = np.ascontiguousarray(
        np.broadcast_to(np.asarray(ln_beta, np.float32), (128, E)))

    nc = _get_nc()
    shared = {
        "dense_w": dense_w.astype(NPBF16),
        "b_col": b.reshape(E, 1),
        "b_col_d": (b / DECAY).reshape(E, 1),
        "w2": w2,
        "id128": np.eye(E, dtype=NPBF16),
        "gamma_rep": gam,
        "beta_rep": bet,
    }
    in_maps = [dict(shared, xT=np.ascontiguousarray(x[bi].T).astype(NPBF16))
               for bi in range(B)]
    res = run_bass_kernel_spmd(nc, in_maps, core_ids=list(range(B)))
    LAST_RESULT = res
    out = np.stack([res.results[bi]["out"] for bi in range(B)]).astype(np.float32)
    return out


if __name__ == "__main__":
    rng = np.random.default_rng(0)
    inputs = {
        "x": rng.standard_normal((B, S, E)).astype(np.float32),
        "dense_w": rng.standard_normal((E, E)).astype(np.float32) * 0.02,
        "dense_b": np.zeros(E, np.float32),
        "concept_map": rng.standard_normal((E, E, E)).astype(np.float32) * 0.02,
        "ln_gamma": np.ones(E, np.float32),
        "ln_beta": np.zeros(E, np.float32),
    }
    out = kernel(**inputs)
    print("out", out.shape, out.dtype, float(np.abs(out).max()))


# revision 10
# speedup vs baseline: 2.9956x; 2.9956x over previous
"""Trainium2 Bass kernel for nn_ConceptLayer (B=8, S=2048, E=128).

out[b,s,c] = LN( einsum('sa,sp,cap->sc', h[b], s_seq[b], W) + h[b] )
  h = x @ dense_w + dense_b
  s_seq = decayed prefix sum of h along s (s_i = (s_{i-1}+h_{i-1})/1.2)

Sharding: data-parallel over batch, one sample per NeuronCore (8 cores).

One-pass PE design: outT[c,t] = sum_a W2[:,a-slice]^T @ (sT * h[:,a]) is
accumulated directly in PSUM across all 128 a-matmuls (plus one identity
matmul adding the residual h^T). No big Y intermediate, no per-a vector
combine. Per a-batch of 4 (x 1024-token supergroup):
  1. flatten h^T rows to one partition, then one stride-0 SBUF DMA per
     64 dst partitions replicates h[t,a] (bf16) to all 128 partitions
  2. DVE builds scaled operands sc[p,(a,t)] = sT[p,t]*hB[p,(a,t)]
     (16-bit tensor_tensor, stride-0 a-dim on the sT operand, 2x mode)
  3. PE: per a one W2-slice load + 2 matmuls (N=512) accumulating into
     per-token-group PSUM banks (alternating banks, no same-bank stall)
  4. per 128-token block: PE transpose back to [t,c] + LayerNorm
     (stats on DVE, sqrt+apply on ACT, gamma/beta on GPSIMD)
Scan stays fp16 (decay 1/1.2 bounds the rounding-error accumulation).
"""

import dataclasses
import os
import sys

import numpy as np

for _p in ("/opt/trn_rl_repo",):
    if _p not in sys.path and os.path.isdir(_p):
        sys.path.insert(0, _p)

import concourse.bass as bass
import concourse.bacc as bacc
import concourse.tile as tile
from concourse import mybir
from concourse.bass_utils import run_bass_kernel_spmd

B, S, E = 8, 2048, 128
DECAY = 1.2
LN_EPS = 1e-3
NSG = 2            # token supergroups
SGW = S // NSG     # 1024 tokens each
NTGL = SGW // 512  # 2 psum groups of 512 per supergroup
NAB = 32           # a-batches of 4
ABW = E // NAB     # 4
F32 = mybir.dt.float32
F16 = mybir.dt.float16
BF16 = mybir.dt.bfloat16
NPF16 = mybir.dt.np(F16)
NPBF16 = mybir.dt.np(BF16)

_CACHE = {}
LAST_RESULT = None  # BassKernelResults of the most recent run (for test.py)


def _build_nc():
    nc = bacc.Bacc(None, target_bir_lowering=False)

    xT_d = nc.declare_dram_parameter("xT", [E, S], BF16, isOutput=False)
    dw_d = nc.declare_dram_parameter("dense_w", [E, E], BF16, isOutput=False)
    bc_d = nc.declare_dram_parameter("b_col", [E, 1], F32, isOutput=False)
    bcd_d = nc.declare_dram_parameter("b_col_d", [E, 1], F32, isOutput=False)
    w2_d = nc.declare_dram_parameter("w2", [E, E * E], BF16, isOutput=False)
    id_d = nc.declare_dram_parameter("id128", [E, E], BF16, isOutput=False)
    gam_d = nc.declare_dram_parameter("gamma_rep", [128, E], F32, isOutput=False)
    bet_d = nc.declare_dram_parameter("beta_rep", [128, E], F32, isOutput=False)
    out_d = nc.declare_dram_parameter("out", [S, E], F32, isOutput=True)

    with tile.TileContext(nc) as tc:
        with (
            tc.tile_pool(name="singles", bufs=1) as singles,
            tc.tile_pool(name="hb", bufs=2) as hb_pool,
            tc.tile_pool(name="sc", bufs=2) as sc_pool,
            tc.tile_pool(name="ln", bufs=2) as ln_pool,
            tc.tile_pool(name="small", bufs=4) as small,
            tc.tile_pool(name="h_ps", bufs=2, space="PSUM") as h_ps,
            tc.tile_pool(name="mm_ps", bufs=1, space="PSUM") as mm_ps,
            tc.tile_pool(name="tr_ps", bufs=2, space="PSUM") as tr_ps,
        ):
            # ---- resident tensors ----
            w2_sb = singles.tile([E, E * E], BF16)
            nc.sync.dma_start(out=w2_sb[:], in_=w2_d[:])
            dw_sb = singles.tile([E, E], BF16)
            nc.sync.dma_start(out=dw_sb[:], in_=dw_d[:])
            id_sb = singles.tile([E, E], BF16)
            nc.sync.dma_start(out=id_sb[:], in_=id_d[:])
            bcol = singles.tile([E, 1], F32)
            nc.sync.dma_start(out=bcol[:], in_=bc_d[:])
            bcold = singles.tile([E, 1], F32)
            nc.sync.dma_start(out=bcold[:], in_=bcd_d[:])
            gam_sb = singles.tile([128, E], F32)
            nc.sync.dma_start(out=gam_sb[:], in_=gam_d[:])
            bet_sb = singles.tile([128, E], F32)
            nc.sync.dma_start(out=bet_sb[:], in_=bet_d[:])
            xT = singles.tile([E, S], BF16)
            nc.sync.dma_start(out=xT[:], in_=xT_d[:])

            eps_t = singles.tile([128, 1], F32)
            nc.vector.memset(eps_t[:], LN_EPS)
            dinv = singles.tile([128, 512], F16)
            nc.vector.memset(dinv[:], 1.0 / DECAY)

            hTc = singles.tile([E, S], BF16)     # h^T (with bias), clean
            hTp = singles.tile([E, S + 1], F16)  # scan feed: col j+1 = (h_j+b)/d
            sTmm = singles.tile([E, S], F16)     # decayed prefix sums s^T
            nc.vector.memset(hTp[:, 0:1], 0.0)

            # ---- phase A: hT = dw^T @ xT (+bias); scan feed ----
            for q in range(4):
                lo, hi = q * 512, (q + 1) * 512
                hp = h_ps.tile([E, 512], F32)
                nc.tensor.matmul(hp[:], dw_sb[:], xT[:, lo:hi],
                                 start=True, stop=True)
                nc.vector.tensor_scalar(
                    hTc[:, lo:hi], hp[:], bcol[:], None, mybir.AluOpType.add)
                nc.scalar.activation(
                    hTp[:, lo + 1:hi + 1], hp[:],
                    mybir.ActivationFunctionType.Identity,
                    bias=bcold[:], scale=1.0 / DECAY)

            # ---- phase B: decay prefix scan -> sT (fp16) ----
            for q in range(4):
                lo, hi = q * 512, (q + 1) * 512
                init = 0.0 if q == 0 else sTmm[:, lo - 1:lo]
                nc.vector.tensor_tensor_scan(
                    sTmm[:, lo:hi], dinv[:], hTp[:, lo:hi], init,
                    mybir.AluOpType.mult, mybir.AluOpType.add)

            # ---- main: single accumulation span over all 2048 tokens ----
            psw = mm_ps.tile([E, S], F32, name="psw")
            ps = [psw[:, t * 512:(t + 1) * 512] for t in range(4)]
            # residual starts each accumulation: outT += Id^T @ hT
            for tgl in range(4):
                nc.tensor.matmul(ps[tgl], id_sb[:],
                                 hTc[:, tgl * 512:(tgl + 1) * 512],
                                 start=True, stop=False,
                                 skip_group_check=True)
            for ab in range(NAB):
                a0 = ab * ABW
                # replicate h[t,a] (a-major flat) to all 128 partitions:
                # flatten into partition 0, then 7 doubling hops
                hB = hb_pool.tile([128, ABW, S], BF16)
                eng = nc.sync if ab % 2 == 0 else nc.gpsimd
                eng.dma_start(out=hB[0:1, :, :], in_=hTc[a0:a0 + ABW, :])
                k = 1
                while k < 128:
                    eng.dma_start(out=hB[k:2 * k, :, :], in_=hB[0:k, :, :])
                    k *= 2
                # scaled operands: sc[p,(a,t)] = sT[p,t] * h[t,a]
                sc = sc_pool.tile([128, ABW, S], BF16)
                in0 = sTmm[:, :]
                in0 = dataclasses.replace(
                    in0, ap=[in0.ap[0], [0, ABW]] + in0.ap[1:])
                nc.vector.tensor_tensor(out=sc[:, :, :], in0=in0,
                                        in1=hB[:, :, :],
                                        op=mybir.AluOpType.mult)
                # PE: per a, one W2 slice load amortized over 4 matmuls
                for i in range(ABW):
                    a = a0 + i
                    for tgl in range(4):
                        last = (ab == NAB - 1) and (i == ABW - 1)
                        nc.tensor.matmul(
                            ps[tgl], w2_sb[:, a * E:(a + 1) * E],
                            sc[:, i, tgl * 512:(tgl + 1) * 512],
                            start=False, stop=last,
                            skip_group_check=True)

            # drain, transpose back, LayerNorm per 128-token block
            for tgl in range(4):
                rT = ln_pool.tile([E, 512], BF16)
                nc.scalar.copy(out=rT[:], in_=ps[tgl])
                for blk in range(4):
                    g = tgl * 4 + blk
                    rtp = tr_ps.tile([128, 128], BF16)
                    nc.tensor.transpose(
                        rtp[:], rT[:, blk * 128:(blk + 1) * 128], id_sb[:])
                    stats = small.tile([128, 6], F32)
                    nc.vector.bn_stats(out=stats[:], in_=rtp[:])
                    mv = small.tile([128, 2], F32)
                    nc.vector.bn_aggr(out=mv[:], in_=stats[:])
                    std = small.tile([128, 1], F32)
                    nc.scalar.activation(
                        out=std[:], in_=mv[:, 1:2],
                        func=mybir.ActivationFunctionType.Sqrt,
                        bias=eps_t[:], scale=1.0)
                    rstd = small.tile([128, 1], F32)
                    nc.vector.reciprocal(out=rstd[:], in_=std[:])
                    nbias = small.tile([128, 1], F32)
                    nc.vector.tensor_scalar(
                        nbias[:], mv[:, 0:1], rstd[:], -1.0,
                        mybir.AluOpType.mult, mybir.AluOpType.mult)
                    nrm = ln_pool.tile([128, E], F32)
                    nc.scalar.activation(
                        out=nrm[:], in_=rtp[:],
                        func=mybir.ActivationFunctionType.Identity,
                        bias=nbias[:], scale=rstd[:])
                    nc.gpsimd.tensor_tensor(out=nrm[:], in0=nrm[:],
                                            in1=gam_sb[:],
                                            op=mybir.AluOpType.mult)
                    nc.gpsimd.tensor_tensor(out=nrm[:], in0=nrm[:],
                                            in1=bet_sb[:],
                                            op=mybir.AluOpType.add)
                    nc.scalar.dma_start(
                        out=out_d[g * 128:(g + 1) * 128, :], in_=nrm[:])

    nc.compile()
    return nc


def _get_nc():
    if "nc" not in _CACHE:
        _CACHE["nc"] = _build_nc()
    return _CACHE["nc"]


def kernel(x, dense_w, dense_b, concept_map, ln_gamma, ln_beta):
    global LAST_RESULT
    x = np.asarray(x, dtype=np.float32)
    dense_w = np.ascontiguousarray(np.asarray(dense_w, dtype=np.float32))
    b = np.asarray(dense_b, dtype=np.float32)
    w2 = np.ascontiguousarray(
        np.transpose(np.asarray(concept_map, dtype=np.float32), (2, 1, 0))
    ).reshape(E, E * E).astype(NPBF16)
    gam = np.ascontiguousarray(
        np.broadcast_to(np.asarray(ln_gamma, np.float32), (128, E)))
    bet = np.ascontiguousarray(
        np.broadcast_to(np.asarray(ln_beta, np.float32), (128, E)))

    nc = _get_nc()
    shared = {
        "dense_w": dense_w.astype(NPBF16),
        "b_col": b.reshape(E, 1),
        "b_col_d": (b / DECAY).reshape(E, 1),
        "w2": w2,
        "id128": np.eye(E, dtype=NPBF16),
        "gamma_rep": gam,
        "beta_rep": bet,
    }
    in_maps = [dict(shared, xT=np.ascontiguousarray(x[bi].T).astype(NPBF16))
               for bi in range(B)]
    res = run_bass_kernel_spmd(nc, in_maps, core_ids=list(range(B)))
    LAST_RESULT = res
    out = np.stack([res.results[bi]["out"] for bi in range(B)]).astype(np.float32)
    return out


if __name__ == "__main__":
    rng = np.random.default_rng(0)
    inputs = {
        "x": rng.standard_normal((B, S, E)).astype(np.float32),
        "dense_w": rng.standard_normal((E, E)).astype(np.float32) * 0.02,
        "dense_b": np.zeros(E, np.float32),
        "concept_map": rng.standard_normal((E, E, E)).astype(np.float32) * 0.02,
        "ln_gamma": np.ones(E, np.float32),
        "ln_beta": np.zeros(E, np.float32),
    }
    out = kernel(**inputs)
    print("out", out.shape, out.dtype, float(np.abs(out).max()))


# revision 11
# speedup vs baseline: 3.6636x; 1.2230x over previous
"""Trainium2 Bass kernel for nn_ConceptLayer (B=8, S=2048, E=128).

out[b,s,c] = LN( einsum('sa,sp,cap->sc', h[b], s_seq[b], W) + h[b] )
  h = x @ dense_w + dense_b
  s_seq = decayed prefix sum of h along s (s_i = (s_{i-1}+h_{i-1})/1.2)

Sharding: data-parallel over batch, one sample per NeuronCore (8 cores).

One-pass PE design: outT[c,t] = sum_a W2[:,a-slice]^T @ (sT * h[:,a]) is
accumulated directly in PSUM across all 128 a-matmuls (plus one identity
matmul adding the residual h^T). No big Y intermediate, no per-a vector
combine. Per a-batch of 4 (x 1024-token supergroup):
  1. flatten h^T rows to one partition, then one stride-0 SBUF DMA per
     64 dst partitions replicates h[t,a] (bf16) to all 128 partitions
  2. DVE builds scaled operands sc[p,(a,t)] = sT[p,t]*hB[p,(a,t)]
     (16-bit tensor_tensor, stride-0 a-dim on the sT operand, 2x mode)
  3. PE: per a one W2-slice load + 2 matmuls (N=512) accumulating into
     per-token-group PSUM banks (alternating banks, no same-bank stall)
  4. per 128-token block: PE transpose back to [t,c] + LayerNorm
     (stats on DVE, sqrt+apply on ACT, gamma/beta on GPSIMD)
Scan stays fp16 (decay 1/1.2 bounds the rounding-error accumulation).
"""

import dataclasses
import os
import sys

import numpy as np

for _p in ("/opt/trn_rl_repo",):
    if _p not in sys.path and os.path.isdir(_p):
        sys.path.insert(0, _p)

import concourse.bass as bass
import concourse.bacc as bacc
import concourse.tile as tile
from concourse import mybir
from concourse.bass_utils import run_bass_kernel_spmd

B, S, E = 8, 2048, 128
DECAY = 1.2
LN_EPS = 1e-3
NSG = 2            # token supergroups
SGW = S // NSG     # 1024 tokens each
NTGL = SGW // 512  # 2 psum groups of 512 per supergroup
NAB = 32           # a-batches of 4
ABW = E // NAB     # 4
F32 = mybir.dt.float32
F16 = mybir.dt.float16
BF16 = mybir.dt.bfloat16
NPF16 = mybir.dt.np(F16)
NPBF16 = mybir.dt.np(BF16)

_CACHE = {}
LAST_RESULT = None  # BassKernelResults of the most recent run (for test.py)


def _build_nc():
    nc = bacc.Bacc(None, target_bir_lowering=False)

    xT_d = nc.declare_dram_parameter("xT", [E, S], BF16, isOutput=False)
    dw_d = nc.declare_dram_parameter("dense_w", [E, E], BF16, isOutput=False)
    bc_d = nc.declare_dram_parameter("b_col", [E, 1], F32, isOutput=False)
    bcd_d = nc.declare_dram_parameter("b_col_d", [E, 1], F32, isOutput=False)
    w2_d = nc.declare_dram_parameter("w2", [E, E * E], BF16, isOutput=False)
    id_d = nc.declare_dram_parameter("id128", [E, E], BF16, isOutput=False)
    gam_d = nc.declare_dram_parameter("gamma_rep", [128, E], F32, isOutput=False)
    bet_d = nc.declare_dram_parameter("beta_rep", [128, E], F32, isOutput=False)
    out_d = nc.declare_dram_parameter("out", [S, E], F32, isOutput=True)

    with tile.TileContext(nc) as tc:
        with (
            tc.tile_pool(name="singles", bufs=1) as singles,
            tc.tile_pool(name="hb", bufs=4) as hb_pool,
            tc.tile_pool(name="sc", bufs=2) as sc_pool,
            tc.tile_pool(name="ln", bufs=2) as ln_pool,
            tc.tile_pool(name="small", bufs=4) as small,
            tc.tile_pool(name="h_ps", bufs=2, space="PSUM") as h_ps,
            tc.tile_pool(name="mm_ps", bufs=1, space="PSUM") as mm_ps,
            tc.tile_pool(name="tr_ps", bufs=2, space="PSUM") as tr_ps,
        ):
            # ---- resident tensors ----
            w2_sb = singles.tile([E, E * E], BF16)
            nc.sync.dma_start(out=w2_sb[:], in_=w2_d[:])
            dw_sb = singles.tile([E, E], BF16)
            nc.sync.dma_start(out=dw_sb[:], in_=dw_d[:])
            id_sb = singles.tile([E, E], BF16)
            nc.sync.dma_start(out=id_sb[:], in_=id_d[:])
            bcol = singles.tile([E, 1], F32)
            nc.sync.dma_start(out=bcol[:], in_=bc_d[:])
            bcold = singles.tile([E, 1], F32)
            nc.sync.dma_start(out=bcold[:], in_=bcd_d[:])
            gam_sb = singles.tile([128, E], F32)
            nc.sync.dma_start(out=gam_sb[:], in_=gam_d[:])
            bet_sb = singles.tile([128, E], F32)
            nc.sync.dma_start(out=bet_sb[:], in_=bet_d[:])
            xT = singles.tile([E, S], BF16)
            nc.sync.dma_start(out=xT[:], in_=xT_d[:])

            eps_t = singles.tile([128, 1], F32)
            nc.vector.memset(eps_t[:], LN_EPS)
            dinv = singles.tile([128, 512], F16)
            nc.vector.memset(dinv[:], 1.0 / DECAY)

            hTc = singles.tile([E, S], BF16)     # h^T (with bias), clean
            hTp = singles.tile([E, S + 1], F16)  # scan feed: col j+1 = (h_j+b)/d
            sTmm = singles.tile([E, S], F16)     # decayed prefix sums s^T
            nc.vector.memset(hTp[:, 0:1], 0.0)

            # ---- phase A: hT = dw^T @ xT (+bias); scan feed ----
            for q in range(4):
                lo, hi = q * 512, (q + 1) * 512
                hp = h_ps.tile([E, 512], F32)
                nc.tensor.matmul(hp[:], dw_sb[:], xT[:, lo:hi],
                                 start=True, stop=True)
                nc.vector.tensor_scalar(
                    hTc[:, lo:hi], hp[:], bcol[:], None, mybir.AluOpType.add)
                nc.scalar.activation(
                    hTp[:, lo + 1:hi + 1], hp[:],
                    mybir.ActivationFunctionType.Identity,
                    bias=bcold[:], scale=1.0 / DECAY)

            # ---- phase B: decay prefix scan -> sT (fp16) ----
            for q in range(4):
                lo, hi = q * 512, (q + 1) * 512
                init = 0.0 if q == 0 else sTmm[:, lo - 1:lo]
                nc.vector.tensor_tensor_scan(
                    sTmm[:, lo:hi], dinv[:], hTp[:, lo:hi], init,
                    mybir.AluOpType.mult, mybir.AluOpType.add)

            # ---- main: single accumulation span over all 2048 tokens ----
            psw = mm_ps.tile([E, S], F32, name="psw")
            ps = [psw[:, t * 512:(t + 1) * 512] for t in range(4)]
            # residual starts each accumulation: outT += Id^T @ hT
            for tgl in range(4):
                nc.tensor.matmul(ps[tgl], id_sb[:],
                                 hTc[:, tgl * 512:(tgl + 1) * 512],
                                 start=True, stop=False,
                                 skip_group_check=True)
            for ab in range(NAB):
                a0 = ab * ABW
                # replicate h[t,a] (a-major flat) to all 128 partitions:
                # flatten into partition 0, then 7 doubling hops
                hB = hb_pool.tile([128, ABW, S], BF16)
                eng = nc.sync if ab % 2 == 0 else nc.gpsimd
                eng.dma_start(out=hB[0:1, :, :], in_=hTc[a0:a0 + ABW, :])
                k = 1
                while k < 128:
                    eng.dma_start(out=hB[k:2 * k, :, :], in_=hB[0:k, :, :])
                    k *= 2
                # scaled operands: sc[p,(a,t)] = sT[p,t] * h[t,a]
                sc = sc_pool.tile([128, ABW, S], BF16)
                in0 = sTmm[:, :]
                in0 = dataclasses.replace(
                    in0, ap=[in0.ap[0], [0, ABW]] + in0.ap[1:])
                nc.vector.tensor_tensor(out=sc[:, :, :], in0=in0,
                                        in1=hB[:, :, :],
                                        op=mybir.AluOpType.mult)
                # PE: per a, one W2 slice load amortized over 4 matmuls
                for i in range(ABW):
                    a = a0 + i
                    for tgl in range(4):
                        last = (ab == NAB - 1) and (i == ABW - 1)
                        nc.tensor.matmul(
                            ps[tgl], w2_sb[:, a * E:(a + 1) * E],
                            sc[:, i, tgl * 512:(tgl + 1) * 512],
                            start=False, stop=last,
                            skip_group_check=True)

            # drain, transpose back, LayerNorm per 128-token block
            for tgl in range(4):
                rT = ln_pool.tile([E, 512], BF16)
                nc.scalar.copy(out=rT[:], in_=ps[tgl])
                for blk in range(4):
                    g = tgl * 4 + blk
                    rtp = tr_ps.tile([128, 128], BF16)
                    nc.tensor.transpose(
                        rtp[:], rT[:, blk * 128:(blk + 1) * 128], id_sb[:])
                    stats = small.tile([128, 6], F32)
                    nc.vector.bn_stats(out=stats[:], in_=rtp[:])
                    mv = small.tile([128, 2], F32)
                    nc.vector.bn_aggr(out=mv[:], in_=stats[:])
                    std = small.tile([128, 1], F32)
                    nc.scalar.activation(
                        out=std[:], in_=mv[:, 1:2],
                        func=mybir.ActivationFunctionType.Sqrt,
                        bias=eps_t[:], scale=1.0)
                    rstd = small.tile([128, 1], F32)
                    nc.vector.reciprocal(out=rstd[:], in_=std[:])
                    nbias = small.tile([128, 1], F32)
                    nc.vector.tensor_scalar(
                        nbias[:], mv[:, 0:1], rstd[:], -1.0,
                        mybir.AluOpType.mult, mybir.AluOpType.mult)
                    nrm = ln_pool.tile([128, E], F32)
                    nc.scalar.activation(
                        out=nrm[:], in_=rtp[:],
                        func=mybir.ActivationFunctionType.Identity,
                        bias=nbias[:], scale=rstd[:])
                    nc.gpsimd.tensor_tensor(out=nrm[:], in0=nrm[:],
                                            in1=gam_sb[:],
                                            op=mybir.AluOpType.mult)
                    nc.gpsimd.tensor_tensor(out=nrm[:], in0=nrm[:],
                                            in1=bet_sb[:],
                                            op=mybir.AluOpType.add)
                    nc.scalar.dma_start(
                        out=out_d[g * 128:(g + 1) * 128, :], in_=nrm[:])

    nc.compile()
    return nc


def _get_nc():
    if "nc" not in _CACHE:
        _CACHE["nc"] = _build_nc()
    return _CACHE["nc"]


def kernel(x, dense_w, dense_b, concept_map, ln_gamma, ln_beta):
    global LAST_RESULT
    x = np.asarray(x, dtype=np.float32)
    dense_w = np.ascontiguousarray(np.asarray(dense_w, dtype=np.float32))
    b = np.asarray(dense_b, dtype=np.float32)
    w2 = np.ascontiguousarray(
        np.transpose(np.asarray(concept_map, dtype=np.float32), (2, 1, 0))
    ).reshape(E, E * E).astype(NPBF16)
    gam = np.ascontiguousarray(
        np.broadcast_to(np.asarray(ln_gamma, np.float32), (128, E)))
    bet = np.ascontiguousarray(
        np.broadcast_to(np.asarray(ln_beta, np.float32), (128, E)))

    nc = _get_nc()
    shared = {
        "dense_w": dense_w.astype(NPBF16),
        "b_col": b.reshape(E, 1),
        "b_col_d": (b / DECAY).reshape(E, 1),
        "w2": w2,
        "id128": np.eye(E, dtype=NPBF16),
        "gamma_rep": gam,
        "beta_rep": bet,
    }
    in_maps = [dict(shared, xT=np.ascontiguousarray(x[bi].T).astype(NPBF16))
               for bi in range(B)]
    res = run_bass_kernel_spmd(nc, in_maps, core_ids=list(range(B)))
    LAST_RESULT = res
    out = np.stack([res.results[bi]["out"] for bi in range(B)]).astype(np.float32)
    return out


if __name__ == "__main__":
    rng = np.random.default_rng(0)
    inputs = {
        "x": rng.standard_normal((B, S, E)).astype(np.float32),
        "dense_w": rng.standard_normal((E, E)).astype(np.float32) * 0.02,
        "dense_b": np.zeros(E, np.float32),
        "concept_map": rng.standard_normal((E, E, E)).astype(np.float32) * 0.02,
        "ln_gamma": np.ones(E, np.float32),
        "ln_beta": np.zeros(E, np.float32),
    }
    out = kernel(**inputs)
    print("out", out.shape, out.dtype, float(np.abs(out).max()))


# revision 17
# speedup vs baseline: 9.0864x; 2.4802x over previous
"""Trainium2 Bass kernel for nn_ConceptLayer (B=8, S=2048, E=128).

out[b,s,c] = LN( einsum('sa,sp,cap->sc', h[b], s_seq[b], W) + h[b] )
  h = x @ dense_w + dense_b
  s_seq = decayed prefix sum of h along s (s_i = (s_{i-1}+h_{i-1})/1.2)

Sharding: data-parallel over batch, one sample per NeuronCore (8 cores).

One-pass PE design: outT[c,t] = sum_a W2[:,a-slice]^T @ (sT * h[:,a]) is
accumulated directly in PSUM across all 128 a-matmuls (plus one identity
matmul adding the residual h^T). No big Y intermediate, no per-a vector
combine. Per a-batch of 4 (x 1024-token supergroup):
  1. flatten h^T rows to one partition, then one stride-0 SBUF DMA per
     64 dst partitions replicates h[t,a] (bf16) to all 128 partitions
  2. DVE builds scaled operands sc[p,(a,t)] = sT[p,t]*hB[p,(a,t)]
     (16-bit tensor_tensor, stride-0 a-dim on the sT operand, 2x mode)
  3. PE: per a one W2-slice load + 2 matmuls (N=512) accumulating into
     per-token-group PSUM banks (alternating banks, no same-bank stall)
  4. per 128-token block: PE transpose back to [t,c] + LayerNorm
     (stats on DVE, sqrt+apply on ACT, gamma/beta on GPSIMD)
Scan stays fp16 (decay 1/1.2 bounds the rounding-error accumulation).
"""

import dataclasses
import os
import sys

import numpy as np

for _p in ("/opt/trn_rl_repo",):
    if _p not in sys.path and os.path.isdir(_p):
        sys.path.insert(0, _p)

import concourse.bass as bass
import concourse.bacc as bacc
import concourse.tile as tile
from concourse import mybir
from concourse.bass_utils import run_bass_kernel_spmd

B, S, E = 8, 2048, 128
DECAY = 1.2
LN_EPS = 1e-3
NSG = 2            # token supergroups
SGW = S // NSG     # 1024 tokens each
NTGL = SGW // 512  # 2 psum groups of 512 per supergroup
NAB = 32           # a-batches of 4
ABW = E // NAB     # 4
F32 = mybir.dt.float32
F16 = mybir.dt.float16
BF16 = mybir.dt.bfloat16
NPF16 = mybir.dt.np(F16)
NPBF16 = mybir.dt.np(BF16)

_CACHE = {}
LAST_RESULT = None  # BassKernelResults of the most recent run (for test.py)


def _build_nc():
    nc = bacc.Bacc(None, target_bir_lowering=False)

    xT_d = nc.declare_dram_parameter("xT", [E, S], BF16, isOutput=False)
    dw_d = nc.declare_dram_parameter("dense_w", [E, E], BF16, isOutput=False)
    bc_d = nc.declare_dram_parameter("b_col", [E, 1], F32, isOutput=False)
    bcd_d = nc.declare_dram_parameter("b_col_d", [E, 1], F32, isOutput=False)
    w2_d = nc.declare_dram_parameter("w2", [E, E * E], BF16, isOutput=False)
    id_d = nc.declare_dram_parameter("id128", [E, E], BF16, isOutput=False)
    gam_d = nc.declare_dram_parameter("gamma_rep", [128, E], F32, isOutput=False)
    bet_d = nc.declare_dram_parameter("beta_rep", [128, E], F32, isOutput=False)
    out_d = nc.declare_dram_parameter("out", [S, E], F32, isOutput=True)
    st_dram = nc.dram_tensor("st_stage", [E, S], F16, kind="Internal")

    with tile.TileContext(nc) as tc:
        with (
            tc.tile_pool(name="singles", bufs=1) as singles,
            tc.tile_pool(name="sc", bufs=4) as sc_pool,
            tc.tile_pool(name="ln", bufs=2) as ln_pool,
            tc.tile_pool(name="small", bufs=4) as small,
            tc.tile_pool(name="h_ps", bufs=2, space="PSUM") as h_ps,
            tc.tile_pool(name="mm_ps", bufs=1, space="PSUM") as mm_ps,
            tc.tile_pool(name="tr_ps", bufs=2, space="PSUM") as tr_ps,
        ):
            # ---- resident tensors ----
            w2_sb = singles.tile([E, E * E], BF16)
            nc.sync.dma_start(out=w2_sb[:], in_=w2_d[:])
            dw_sb = singles.tile([E, E], BF16)
            nc.sync.dma_start(out=dw_sb[:], in_=dw_d[:])
            id_sb = singles.tile([E, E], BF16)
            nc.sync.dma_start(out=id_sb[:], in_=id_d[:])
            bcol = singles.tile([E, 1], F32)
            nc.sync.dma_start(out=bcol[:], in_=bc_d[:])
            bcold = singles.tile([E, 1], F32)
            nc.sync.dma_start(out=bcold[:], in_=bcd_d[:])
            gam_sb = singles.tile([128, E], F32)
            nc.sync.dma_start(out=gam_sb[:], in_=gam_d[:])
            bet_sb = singles.tile([128, E], F32)
            nc.sync.dma_start(out=bet_sb[:], in_=bet_d[:])
            xT = singles.tile([E, S], BF16)
            nc.sync.dma_start(out=xT[:], in_=xT_d[:])

            eps_t = singles.tile([128, 1], F32)
            nc.vector.memset(eps_t[:], LN_EPS)
            dinv = singles.tile([128, 512], F16)
            nc.vector.memset(dinv[:], 1.0 / DECAY)

            hTc = singles.tile([E, S], BF16)     # h^T (with bias), clean
            hTp = singles.tile([E, S + 1], F16)  # scan feed: col j+1 = (h_j+b)/d
            sTmm = singles.tile([E, S], F16)     # decayed prefix sums s^T
            nc.vector.memset(hTp[:, 0:1], 0.0)

            # ---- phase A: hT = dw^T @ xT (+bias); scan feed ----
            for q in range(4):
                lo, hi = q * 512, (q + 1) * 512
                hp = h_ps.tile([E, 512], F32)
                nc.tensor.matmul(hp[:], dw_sb[:], xT[:, lo:hi],
                                 start=True, stop=True)
                nc.vector.tensor_scalar(
                    hTc[:, lo:hi], hp[:], bcol[:], None, mybir.AluOpType.add)
                nc.scalar.activation(
                    hTp[:, lo + 1:hi + 1], hp[:],
                    mybir.ActivationFunctionType.Identity,
                    bias=bcold[:], scale=1.0 / DECAY)

            # ---- phase B: decay prefix scan -> sT (fp16) ----
            for q in range(4):
                lo, hi = q * 512, (q + 1) * 512
                init = 0.0 if q == 0 else sTmm[:, lo - 1:lo]
                nc.vector.tensor_tensor_scan(
                    sTmm[:, lo:hi], dinv[:], hTp[:, lo:hi], init,
                    mybir.AluOpType.mult, mybir.AluOpType.add)

            # ---- replicate factors once (dual replication, ~16MB total) ----
            # chunk rows r = p_idx*16 + a_idx: a = Ag*16 + r%16, p = pg*8 + r//16
            # sTrep[pg][r, t] = sT[pg*8 + r//16, t]: each sT row tiled to 16
            # consecutive partitions, via a DRAM-staged step-0 read
            nc.sync.dma_start(out=st_dram[:], in_=sTmm[:, :])
            strep = []
            for pg in range(16):
                sr = singles.tile([128, S], F16, name=f"sTrep{pg}")
                strep.append(sr)
                src = st_dram[pg * 8:(pg + 1) * 8, :]
                bsrc = dataclasses.replace(
                    src, ap=[src.ap[0], [0, 16], [1, S]])
                eng = nc.sync if pg % 2 == 0 else nc.gpsimd
                eng.dma_start(out=sr[:], in_=bsrc)
            # hBA[Ag][r, t] = h[t, Ag*16 + r%16]: 16 rows tiled 8x
            # vertically (contiguous doubling ladder)
            hba = []
            for Ag in range(8):
                hb = singles.tile([128, S], BF16, name=f"hBA{Ag}")
                hba.append(hb)
                eng = nc.sync if Ag % 2 == 0 else nc.gpsimd
                eng.dma_start(out=hb[0:16, :],
                              in_=hTc[Ag * 16:(Ag + 1) * 16, :])
                k = 16
                while k < 128:
                    eng.dma_start(out=hb[k:2 * k, :], in_=hb[0:k, :])
                    k *= 2

            # ---- main: single accumulation span over all 2048 tokens ----
            psw = mm_ps.tile([E, S], F32, name="psw")
            ps = [psw[:, t * 512:(t + 1) * 512] for t in range(4)]
            # residual starts each accumulation: outT += Id^T @ hT
            for tgl in range(4):
                nc.tensor.matmul(ps[tgl], id_sb[:],
                                 hTc[:, tgl * 512:(tgl + 1) * 512],
                                 start=True, stop=False,
                                 skip_group_check=True)
            for Ag in range(8):
                for pg in range(16):
                    ci = Ag * 16 + pg
                    sc = sc_pool.tile([128, S], BF16)
                    nc.vector.tensor_tensor(out=sc[:], in0=strep[pg][:],
                                            in1=hba[Ag][:],
                                            op=mybir.AluOpType.mult)
                    for tgl in range(4):
                        last = ci == 127
                        nc.tensor.matmul(
                            ps[tgl], w2_sb[:, ci * E:(ci + 1) * E],
                            sc[:, tgl * 512:(tgl + 1) * 512],
                            start=False, stop=last,
                            skip_group_check=True)

            # drain, transpose back, LayerNorm per 128-token block
            for tgl in range(4):
                rT = ln_pool.tile([E, 512], BF16)
                nc.scalar.copy(out=rT[:], in_=ps[tgl])
                for blk in range(4):
                    g = tgl * 4 + blk
                    rtp = tr_ps.tile([128, 128], BF16)
                    nc.tensor.transpose(
                        rtp[:], rT[:, blk * 128:(blk + 1) * 128], id_sb[:])
                    stats = small.tile([128, 6], F32)
                    nc.vector.bn_stats(out=stats[:], in_=rtp[:])
                    mv = small.tile([128, 2], F32)
                    nc.vector.bn_aggr(out=mv[:], in_=stats[:])
                    std = small.tile([128, 1], F32)
                    nc.scalar.activation(
                        out=std[:], in_=mv[:, 1:2],
                        func=mybir.ActivationFunctionType.Sqrt,
                        bias=eps_t[:], scale=1.0)
                    rstd = small.tile([128, 1], F32)
                    nc.vector.reciprocal(out=rstd[:], in_=std[:])
                    nbias = small.tile([128, 1], F32)
                    nc.vector.tensor_scalar(
                        nbias[:], mv[:, 0:1], rstd[:], -1.0,
                        mybir.AluOpType.mult, mybir.AluOpType.mult)
                    nrm = ln_pool.tile([128, E], F32)
                    nc.scalar.activation(
                        out=nrm[:], in_=rtp[:],
                        func=mybir.ActivationFunctionType.Identity,
                        bias=nbias[:], scale=rstd[:])
                    nc.gpsimd.tensor_tensor(out=nrm[:], in0=nrm[:],
                                            in1=gam_sb[:],
                                            op=mybir.AluOpType.mult)
                    nc.gpsimd.tensor_tensor(out=nrm[:], in0=nrm[:],
                                            in1=bet_sb[:],
                                            op=mybir.AluOpType.add)
                    nc.scalar.dma_start(
                        out=out_d[g * 128:(g + 1) * 128, :], in_=nrm[:])

    nc.compile()
    return nc


def _get_nc():
    if "nc" not in _CACHE:
        _CACHE["nc"] = _build_nc()
    return _CACHE["nc"]


def kernel(x, dense_w, dense_b, concept_map, ln_gamma, ln_beta):
    global LAST_RESULT
    x = np.asarray(x, dtype=np.float32)
    dense_w = np.ascontiguousarray(np.asarray(dense_w, dtype=np.float32))
    b = np.asarray(dense_b, dtype=np.float32)
    # chunk layout: rows r=(p_idx*16+a_idx), cols (ci=Ag*16+pg)*128+c with
    # a=Ag*16+a_idx, p=pg*8+p_idx  ->  [p_idx, a_idx, Ag, pg, c]
    w4 = np.transpose(np.asarray(concept_map, dtype=np.float32), (1, 2, 0))
    w2 = np.ascontiguousarray(
        w4.reshape(8, 16, 16, 8, E).transpose(3, 1, 0, 2, 4)
    ).reshape(E, E * E).astype(NPBF16)
    gam = np.ascontiguousarray(
        np.broadcast_to(np.asarray(ln_gamma, np.float32), (128, E)))
    bet = np.ascontiguousarray(
        np.broadcast_to(np.asarray(ln_beta, np.float32), (128, E)))

    nc = _get_nc()
    shared = {
        "dense_w": dense_w.astype(NPBF16),
        "b_col": b.reshape(E, 1),
        "b_col_d": (b / DECAY).reshape(E, 1),
        "w2": w2,
        "id128": np.eye(E, dtype=NPBF16),
        "gamma_rep": gam,
        "beta_rep": bet,
    }
    in_maps = [dict(shared, xT=np.ascontiguousarray(x[bi].T).astype(NPBF16))
               for bi in range(B)]
    res = run_bass_kernel_spmd(nc, in_maps, core_ids=list(range(B)))
    LAST_RESULT = res
    out = np.stack([res.results[bi]["out"] for bi in range(B)]).astype(np.float32)
    return out


if __name__ == "__main__":
    rng = np.random.default_rng(0)
    inputs = {
        "x": rng.standard_normal((B, S, E)).astype(np.float32),
        "dense_w": rng.standard_normal((E, E)).astype(np.float32) * 0.02,
        "dense_b": np.zeros(E, np.float32),
        "concept_map": rng.standard_normal((E, E, E)).astype(np.float32) * 0.02,
        "ln_gamma": np.ones(E, np.float32),
        "ln_beta": np.zeros(E, np.float32),
    }
    out = kernel(**inputs)
    print("out", out.shape, out.dtype, float(np.abs(out).max()))
